# revision 1
# baseline (speedup 1.0000x reference)
"""Performer (FAVOR+) attention TRN2 kernel.

Sharding: 8 cores = 2 batches x 4 head-groups (4 heads each).
Core c: batch b = c // 4, heads 4*(c%4) .. 4*(c%4)+3.
Each core computes its 4 heads' full pipeline from a host-pre-transposed
x^T and a column/row slice of W_qkv / W_out; the host sums the 4 partial
output projections per batch (bf16 device output, f32 accumulate).

Math (per head, exact rewrite of the reference):
  u_k = k @ pmT, kf_raw = exp(+-u_k - diag_k)      (no stab, no 1/16)
  kv_raw = kf_raw^T @ [v | 1]                      ([256, 65], col 64 = ksum)
  u_q = q @ pmT, eqf = exp(+-u_q - 4ln2)           (fp8, bias for range)
  o_aug[n, a] = sum_f eqf[f, n] * kv8[f, a]        (col 64 = z, fp8 DoubleRow)
  out = o * r,  r = 1/(z + eps'),  y = out @ W_out
  eps' = exp(diag_q + stab_q + stab_k + ln(256e-6) - 4ln2) * s_h
The q-side factors exp(-diag_q - stab_q)/16 and k-side exp(-stab_k)/16
cancel in the o/z ratio and are reintroduced exactly through eps'. The
eq fp8 bias 2^-4 and the per-head kv fp8 scale s_h = 120/absmax(kv)
also cancel in o/z, entering only via eps'.

Schedule: pass 1 streams x once, block(512 pos)-wise: k^T projection
first (it gates the critical chain sq_k -> diag_k -> edk -> vaug -> KV
accumulation), v and q^T projections fill the PE meanwhile; q-side
stats (stab_q/diag_q, for eps' only) trail each block. Pass 2 per
block: u_q^T -> eq (fp8) -> attention o_aug (fp8 DoubleRow matmuls) ->
rescale by r in natural layout -> bf16 transpose -> y projection ->
bf16 y DMA. PSUM rings are tagged per producer so unrelated stages
don't share WAR chains. Projections/u/KV in float32r (full PE rate at
free >= 256); attention + output projection tails in bf16/fp8e4.
"""
import sys

if "/opt/trn_rl_repo" not in sys.path:
    sys.path.insert(0, "/opt/trn_rl_repo")

from contextlib import ExitStack

import ml_dtypes
import numpy as np

import concourse.bass as bass
import concourse.bacc as bacc_mod
import concourse.mybir as mybir
import concourse.tile as tile
from concourse.bass import ds
from concourse.bass_utils import run_bass_kernel_spmd
from concourse.masks import make_identity

F32 = mybir.dt.float32
F32R = mybir.dt.float32r
BF16 = mybir.dt.bfloat16
FP8 = mybir.dt.float8e4
DR = mybir.MatmulPerfMode.DoubleRow
EXP = mybir.ActivationFunctionType.Exp
AX = mybir.AxisListType.X
ADD = mybir.AluOpType.add
MULT = mybir.AluOpType.mult

MMLAB = {}        # instruction name -> site label (for analyze.py)

D = 1024          # model dim
JL = 256          # local j (4 heads * 64)
KO = 8            # d-tiles
LNEPS = float(np.log(256.0e-6))   # 2*ln16 + ln(1e-6)
EQB = float(np.log(2.0 ** -4))    # eq fp8 range bias (cancels via eps scale)


def _emit(tc, nc, N, tens):
    NT = N // 128
    NB = N // 512

    def MM(label, *args, **kw):
        i = nc.tensor.matmul(*args, **kw)
        MMLAB[i.ins.name] = label
        return i

    def TR(label, **kw):
        i = nc.tensor.transpose(**kw)
        MMLAB[i.ins.name] = label
        return i
    xT, wq, wk, wv, pm2, wout, onesbd, y = tens

    with ExitStack() as ctx:
        consts = ctx.enter_context(tc.tile_pool(name="consts", bufs=1))
        big = ctx.enter_context(tc.tile_pool(name="big", bufs=1))
        stats = ctx.enter_context(tc.tile_pool(name="stats", bufs=1))

        wq_sb = consts.tile([128, KO, JL], F32R)
        wk_sb = consts.tile([128, KO, JL], F32R)
        wv_sb = consts.tile([128, KO, JL], F32R)
        pm2_sb = consts.tile([128, 2, 256], F32R)
        wout_sb = consts.tile([128, 2, D], BF16)
        onesbd_sb = consts.tile([128, 2], F32R)
        ident = consts.tile([128, 128], F32)
        make_identity(nc, ident)
        ident_bf = consts.tile([128, 128], BF16)
        make_identity(nc, ident_bf)


        kv_sb = consts.tile([128, 2, 4, 65], FP8)       # [f, fh, h, d'|1] scaled
        kv_acc = consts.tile([65, 2, 2, 256], F32)      # [d'|1, jo, hh, f]
        nc.vector.memset(kv_acc, 0.0)

        qT_sb = big.tile([128, 2, N], F32R, tag="qT")

        diagq_nat = stats.tile([128, NT, 4], F32)
        diagk_nat = stats.tile([128, NT, 4], F32)
        edk_nat = stats.tile([128, NT, 4], F32)          # exp(-diag_k)
        stabq_nat = stats.tile([128, NT, 4], F32)
        maxk_all = stats.tile([128, NT, 4], F32)
        epsq_nat = stats.tile([128, NT, 4], F32)
        maxk4 = stats.tile([128, 4], F32)
        stabk_bc = stats.tile([128, 4], F32)
        epsb = stats.tile([128, 1], F32)
        nc.vector.memset(epsb, LNEPS + EQB)
        zerob = stats.tile([128, 1], F32)
        nc.vector.memset(zerob, 0.0)
        eqbb = stats.tile([128, 1], F32)
        nc.vector.memset(eqbb, EQB)
        m2 = stats.tile([65, 4], F32)
        m2r = stats.tile([65, 4], F32)
        s_h = stats.tile([65, 4], F32)
        s_bc = stats.tile([128, 4], F32)

        # ------------- PASS 1 (fused): k-side critical chain first -------------
        xTh = xT.rearrange("(ko p) n -> p ko n", p=128)
        with tc.tile_pool(name="xload", bufs=7) as xpool, \
             tc.tile_pool(name="ktb", bufs=4) as ktpool, \
             tc.tile_pool(name="vab", bufs=4) as vapool, \
             tc.tile_pool(name="sqp", bufs=4) as sqpool, \
             tc.tile_pool(name="kfp", bufs=8) as kfpool, \
             tc.tile_pool(name="ps1", bufs=1, space="PSUM") as ps1:
            wqh = wq.rearrange("(ko p) j -> p ko j", p=128)
            wkh = wk.rearrange("(ko p) j -> p ko j", p=128)
            wvh = wv.rearrange("(ko p) j -> p ko j", p=128)
            # wk first (k-side gates everything), interleaved with x block 0;
            # first chunks minimal so matmul ko=0 starts asap
            nc.scalar.dma_start(out=wk_sb[:, 0:1, :], in_=wkh[:, 0:1, :])
            xb_pre = []
            for half in range(2):
                xbp = xpool.tile([128, 4, 512], F32R, tag="xb")
                xb_pre.append(xbp)
            nc.sync.dma_start(out=xb_pre[0][:, 0:1, :], in_=xTh[:, 0:1, ds(0, 512)])
            nc.scalar.dma_start(out=wk_sb[:, 1:2, :], in_=wkh[:, 1:2, :])
            nc.sync.dma_start(out=xb_pre[0][:, 1:2, :], in_=xTh[:, 1:2, ds(0, 512)])
            nc.scalar.dma_start(out=wk_sb[:, 2:4, :], in_=wkh[:, 2:4, :])
            nc.sync.dma_start(out=xb_pre[0][:, 2:4, :], in_=xTh[:, 2:4, ds(0, 512)])
            nc.scalar.dma_start(out=wk_sb[:, 4:8, :], in_=wkh[:, 4:8, :])
            nc.sync.dma_start(out=xb_pre[1][:, 0:2, :], in_=xTh[:, 4:6, ds(0, 512)])
            nc.sync.dma_start(out=xb_pre[1][:, 2:4, :], in_=xTh[:, 6:8, ds(0, 512)])
            nc.sync.dma_start(out=onesbd_sb, in_=onesbd[:, :])
            nc.sync.dma_start(out=pm2_sb, in_=pm2.rearrange("j p f -> p j f"))
            nc.scalar.dma_start(out=wv_sb[:, 0:4, :], in_=wvh[:, 0:4, :])
            nc.scalar.dma_start(out=wv_sb[:, 4:8, :], in_=wvh[:, 4:8, :])
            nc.scalar.dma_start(out=wq_sb[:, 0:4, :], in_=wqh[:, 0:4, :])
            nc.scalar.dma_start(out=wq_sb[:, 4:8, :], in_=wqh[:, 4:8, :])
            kvs = kfpool.tile([65, 2, 2, 256], BF16, tag="kvs", bufs=1)
            for blk in range(NB):
                nb = ds(blk * 512, 512)
                if blk == 0:
                    xbs = tuple(xb_pre)
                else:
                    xb_lo = xpool.tile([128, 4, 512], F32R, tag="xb")
                    nc.sync.dma_start(out=xb_lo, in_=xTh[:, 0:4, nb])
                    xb_hi = xpool.tile([128, 4, 512], F32R, tag="xb")
                    nc.scalar.dma_start(out=xb_hi, in_=xTh[:, 4:8, nb])
                    xbs = (xb_lo, xb_hi)

                kT_blk = ktpool.tile([128, 2, 512], F32R, tag="ktb")
                for jo in range(2):
                    pt = ps1.tile([128, 512], F32, tag="qk", bufs=2)
                    for ko in range(KO):
                        MM("qk", pt, wk_sb[:, ko, ds(jo * 128, 128)],
                           xbs[ko // 4][:, ko % 4, :],
                           start=(ko == 0), stop=(ko == KO - 1))
                    nc.scalar.copy(out=kT_blk[:, jo, :], in_=pt)
                # k-side diag: gates edk -> vaug -> kv
                sqk = []
                for jo in range(2):
                    sq = sqpool.tile([128, 512], F32R, tag="sq")
                    nc.gpsimd.tensor_mul(out=sq, in0=kT_blk[:, jo, :].bitcast(F32),
                                         in1=kT_blk[:, jo, :].bitcast(F32))
                    sqk.append(sq)
                pdgk = ps1.tile([128, 2, 4, 2], F32, tag="uq", bufs=1)
                for jo in range(2):
                    for nt in range(4):
                        MM("diag", pdgk[:, jo, nt, :], sqk[jo][:, ds(nt * 128, 128)],
                           onesbd_sb, start=True, stop=True)
                nc.any.tensor_copy(
                    out=diagk_nat[:, ds(blk * 4, 4), :].rearrange(
                        "p t (jo u) -> p jo t u", jo=2),
                    in_=pdgk)
                nc.scalar.activation(out=edk_nat[:, ds(blk * 4, 4), :],
                                     in_=diagk_nat[:, ds(blk * 4, 4), :],
                                     func=EXP, bias=zerob, scale=-1.0)
                # v projection (PE filler while Pool computes sq_k)
                pv = ps1.tile([128, 4, 256], F32, tag="pv", bufs=1)
                for nt in range(4):
                    for ko in range(KO):
                        MM("v", pv[:, nt, :],
                           xbs[ko // 4][:, ko % 4, ds(nt * 128, 128)],
                           wv_sb[:, ko, :],
                           start=(ko == 0), stop=(ko == KO - 1))
                # q^T projection (more PE filler)
                for jo in range(2):
                    pt = ps1.tile([128, 512], F32, tag="qk", bufs=2)
                    for ko in range(KO):
                        MM("qk", pt, wq_sb[:, ko, ds(jo * 128, 128)],
                           xbs[ko // 4][:, ko % 4, :],
                           start=(ko == 0), stop=(ko == KO - 1))
                    nc.scalar.copy(out=qT_sb[:, jo, nb], in_=pt)
                # vaug = [v * edk | edk]
                vaug = vapool.tile([128, 4, 4, 65], F32R, tag="va")
                for nt in range(4):
                    t = blk * 4 + nt
                    edb = bass.AP(tensor=edk_nat.tensor,
                                  offset=edk_nat[:, t, :].offset,
                                  ap=list(edk_nat[:, t, :].ap[:-1])
                                  + [list(edk_nat[:, t, :].ap[-1]), [0, 64]])
                    nc.vector.tensor_tensor(
                        out=vaug[:, nt, :, 0:64],
                        in0=pv[:, nt, :].rearrange("p (h e) -> p h e", h=4),
                        in1=edb, op=MULT)
                    nc.any.tensor_copy(
                        out=vaug[:, nt, :, 64:65],
                        in_=edk_nat[:, t, :].rearrange("p (h o) -> p h o", o=1))
                # u_k -> kf -> per-head KV
                for jo in range(2):
                    kfs = {}
                    for hf in range(2):
                        puk = ps1.tile([128, 2, 256], F32, tag="uk", bufs=2)
                        for i in range(2):
                            nt = hf * 2 + i
                            MM("uk", puk[:, i, :],
                               kT_blk[:, jo, ds(nt * 128, 128)],
                               pm2_sb[:, jo, :], start=True, stop=True)
                        kf4 = kfpool.tile([128, 2, 2, 256], F32R, tag="kf")
                        puk4 = puk.rearrange("p i (hh f) -> p i hh f", hh=2)
                        nc.scalar.activation(
                            out=kf4[:, :, :, 0:128], in_=puk4,
                            func=EXP, bias=zerob, scale=1.0)
                        nc.scalar.activation(
                            out=kf4[:, :, :, 128:256], in_=puk4,
                            func=EXP, bias=zerob, scale=-1.0)
                        nc.vector.reduce_max(
                            out=maxk_all[:, ds(blk * 4 + hf * 2, 2), ds(jo * 2, 2)],
                            in_=puk.rearrange("p t (h f) -> p t h f", h=2), axis=AX)
                        kfs[hf] = kf4
                    pkv = ps1.tile([65, 2, 256], F32, tag="kv", bufs=1)
                    for hh in range(2):
                        h = jo * 2 + hh
                        for nt in range(4):
                            MM("kv", pkv[:, hh, :],
                               vaug[:, nt, h, :],
                               kfs[nt // 2][:, nt % 2, hh, :],
                               start=(nt == 0), stop=(nt == 3))
                    if blk == NB - 1:
                        nc.vector.tensor_tensor(
                            out=kvs[:, jo, :, :], in0=kv_acc[:, jo, :, :],
                            in1=pkv, op=ADD)
                    else:
                        nc.vector.tensor_tensor(
                            out=kv_acc[:, jo, :, :], in0=kv_acc[:, jo, :, :],
                            in1=pkv, op=ADD)
                # q-side stats (feed eps' in pass 2 only): emitted last
                for jo in range(2):
                    for hf in range(2):
                        # last block: reuse the then-idle qk ring for depth
                        uqtag = "qk" if blk == NB - 1 else "uq"
                        uqbufs = 2 if blk == NB - 1 else 1
                        puq = ps1.tile([128, 2, 256], F32, tag=uqtag, bufs=uqbufs)
                        for i in range(2):
                            nt = hf * 2 + i
                            MM("uq", puq[:, i, :],
                               qT_sb[:, jo, ds(blk * 512 + nt * 128, 128)],
                               pm2_sb[:, jo, :], start=True, stop=True)
                        nc.vector.reduce_max(
                            out=stabq_nat[:, ds(blk * 4 + hf * 2, 2), ds(jo * 2, 2)],
                            in_=puq.rearrange("p t (h f) -> p t h f", h=2), axis=AX)
                for jo in range(2):
                    sq = sqpool.tile([128, 512], F32R, tag="sq")
                    nc.gpsimd.tensor_mul(out=sq,
                                         in0=qT_sb[:, jo, nb].bitcast(F32),
                                         in1=qT_sb[:, jo, nb].bitcast(F32))
                    pdg = ps1.tile([128, 4, 2], F32, tag="uq", bufs=1)
                    for nt in range(4):
                        MM("diag", pdg[:, nt, :], sq[:, ds(nt * 128, 128)],
                           onesbd_sb, start=True, stop=True)
                    nc.any.tensor_copy(
                        out=diagq_nat[:, ds(blk * 4, 4), ds(jo * 2, 2)], in_=pdg)
                nc.vector.tensor_add(
                    out=epsq_nat[:, ds(blk * 4, 4), :],
                    in0=diagq_nat[:, ds(blk * 4, 4), :],
                    in1=stabq_nat[:, ds(blk * 4, 4), :])
                nc.scalar.activation(
                    out=epsq_nat[:, ds(blk * 4, 4), :],
                    in_=epsq_nat[:, ds(blk * 4, 4), :],
                    func=EXP, bias=epsb, scale=1.0)
            # ---- finalize: per-head kv fp8 scale, kv transpose, stab_k, eps ----
            from concourse import bass_isa
            for jo in range(2):
                nc.vector.tensor_reduce(
                    out=m2[:, ds(jo * 2, 2)].rearrange("p (h o) -> p h o", o=1),
                    in_=kvs[:, jo, :, :], axis=AX,
                    op=mybir.AluOpType.max, apply_absolute_value=True)
            nc.gpsimd.partition_all_reduce(m2r, m2, channels=65,
                                           reduce_op=bass_isa.ReduceOp.max)
            # s_h = 120 / absmax
            nc.vector.reciprocal(out=m2, in_=m2r)
            nc.vector.tensor_scalar(out=s_h, in0=m2, scalar1=120.0, scalar2=None,
                                    op0=MULT)
            nc.gpsimd.partition_broadcast(s_bc, s_h[0:1, :], channels=128)
            for h in range(4):
                jo, hh = h // 2, h % 2
                for fh in range(2):
                    pk = ps1.tile([128, 65], BF16,
                                  tag="uk" if fh == 0 else "qk", bufs=2)
                    TR("kvT", out=pk,
                       in_=kvs[:, jo, hh, ds(fh * 128, 128)],
                       identity=ident_bf[0:65, 0:65])
                    nc.vector.tensor_scalar(
                        out=kv_sb[:, fh, h, :], in0=pk,
                        scalar1=s_bc[:, h:h + 1], scalar2=None, op0=MULT)
            nc.vector.reduce_max(out=maxk4,
                                 in_=maxk_all.rearrange("p t h -> p h t"), axis=AX)
            from concourse import bass_isa
            nc.gpsimd.partition_all_reduce(stabk_bc, maxk4, channels=128,
                                           reduce_op=bass_isa.ReduceOp.max)
            # eps' tail: epsq(partial, per block) * exp(stab_k) * s_h
            esk = stats.tile([128, 4], F32)
            nc.scalar.activation(out=esk, in_=stabk_bc, func=EXP,
                                 bias=zerob, scale=1.0)
            nc.vector.tensor_tensor(out=esk, in0=esk, in1=s_bc, op=MULT)
            eskb = bass.AP(tensor=esk.tensor, offset=esk.offset,
                           ap=[list(esk.ap[0]), [0, NT], list(esk.ap[1])])
            nc.vector.tensor_tensor(out=epsq_nat, in0=epsq_nat, in1=eskb, op=MULT)

        nc.sync.dma_start(out=wout_sb, in_=wout.rearrange("(jo p) d -> p jo d", p=128))
        # ------------- PASS 2: q features, attention (natural), output -------------
        with tc.tile_pool(name="otp", bufs=6) as otpool, \
             tc.tile_pool(name="eqp", bufs=8) as eqpool, \
             tc.tile_pool(name="osc", bufs=8) as opool, \
             tc.tile_pool(name="rrp", bufs=8) as rpool, \
             tc.tile_pool(name="ysb", bufs=10) as ypool, \
             tc.tile_pool(name="p2q", bufs=2, space="PSUM") as psQ, \
             tc.tile_pool(name="p2o", bufs=2, space="PSUM") as psO, \
             tc.tile_pool(name="p2t", bufs=2, space="PSUM") as psT, \
             tc.tile_pool(name="p2y", bufs=2, space="PSUM") as psY:
            pending_y = [None]
            for blk in range(NB):
                nb = ds(blk * 512, 512)
                oT_blk = otpool.tile([128, 2, 512], BF16, tag="ot")
                for h in range(4):
                    if h == 1 and pending_y[0] is not None:
                        pending_y[0]()
                        pending_y[0] = None
                    jo, hh = h // 2, h % 2
                    pq = psQ.tile([128, 512], F32, tag="puT")
                    MM("pq", pq, pm2_sb[:, jo, ds(hh * 128, 128)],
                       qT_sb[:, jo, nb], start=True, stop=True)
                    eq = eqpool.tile([128, 2, 512], FP8, tag="eq")
                    nc.scalar.activation(out=eq[:, 0, :], in_=pq, func=EXP,
                                         bias=eqbb, scale=1.0)
                    nc.scalar.activation(out=eq[:, 1, :], in_=pq, func=EXP,
                                         bias=eqbb, scale=-1.0)
                    po = psO.tile([128, 4, 65], F32, tag="po")
                    for nt in range(4):
                        MM("po", po[:, nt, :],
                           eq[:, :, ds(nt * 128, 128)],
                           kv_sb[:, :, h, :],
                           start=True, stop=True, perf_mode=DR)
                    rr = rpool.tile([128, 4], F32, tag="rr")
                    nc.vector.tensor_tensor(
                        out=rr.rearrange("p (t o) -> p t o", o=1),
                        in0=po[:, :, 64:65],
                        in1=epsq_nat[:, ds(blk * 4, 4), h:h + 1], op=ADD)
                    rr2 = rpool.tile([128, 4], F32, tag="rr2")
                    nc.vector.reciprocal(out=rr2, in_=rr)
                    osc = opool.tile([128, 4, 64], BF16, tag="osc")
                    rrb = bass.AP(tensor=rr2.tensor, offset=rr2.offset,
                                  ap=list(rr2.ap[:-1])
                                  + [list(rr2.ap[-1]), [0, 64]])
                    nc.vector.tensor_tensor(out=osc, in0=po[:, :, 0:64],
                                            in1=rrb, op=MULT)
                    pot = psT.tile([64, 4, 128], BF16, tag="pot")
                    for nt in range(4):
                        TR("oT", out=pot[:, nt, :], in_=osc[:, nt, :],
                           identity=ident_bf)
                    nc.vector.tensor_copy(
                        out=oT_blk[ds(hh * 64, 64), jo, :],
                        in_=pot.rearrange("p t f -> p (t f)"))
                # y = oT.T @ wout: emitted after the next block's first
                # head so its latency-bound chain outranks the y burst
                def _emit_y(blk=blk, oT_blk=oT_blk):
                    for nt in range(4):
                        t = blk * 4 + nt
                        for dch in range(2):
                            py = psY.tile([128, 512], F32, tag="py")
                            for jo in range(2):
                                MM("y", py, oT_blk[:, jo, ds(nt * 128, 128)],
                                   wout_sb[:, jo, ds(dch * 512, 512)],
                                   start=(jo == 0), stop=(jo == 1))
                            ysb = ypool.tile([128, 512], BF16, tag="ysb")
                            nc.any.tensor_copy(out=ysb, in_=py)
                            nc.sync.dma_start(
                                out=y[ds(t * 128, 128), ds(dch * 512, 512)],
                                in_=ysb)
                pending_y[0] = _emit_y
            pending_y[0]()


def build(N):
    nc = bacc_mod.Bacc("TRN2", target_bir_lowering=False)
    xT = nc.dram_tensor("xT", [D, N], F32R, kind="ExternalInput")
    wq = nc.dram_tensor("wq", [D, JL], F32R, kind="ExternalInput")
    wk = nc.dram_tensor("wk", [D, JL], F32R, kind="ExternalInput")
    wv = nc.dram_tensor("wv", [D, JL], F32R, kind="ExternalInput")
    pm2 = nc.dram_tensor("pm2", [2, 128, 256], F32R, kind="ExternalInput")
    wout = nc.dram_tensor("wout", [JL, D], BF16, kind="ExternalInput")
    onesbd = nc.dram_tensor("onesbd", [128, 2], F32R, kind="ExternalInput")
    y = nc.dram_tensor("y", [N, D], BF16, kind="ExternalOutput")
    with tile.TileContext(nc) as tc:
        _emit(tc, nc, N, (xT, wq, wk, wv, pm2, wout, onesbd, y))
    nc.compile()
    return nc


_NC_CACHE = {}


def _get_nc(N):
    if N not in _NC_CACHE:
        _NC_CACHE[N] = build(N)
    return _NC_CACHE[N]


def make_in_maps(x, W_qkv, W_out, proj):
    B, N, D_ = x.shape
    in_maps = []
    onesbd = np.zeros((128, 2), dtype=np.float32)
    onesbd[0:64, 0] = 0.5
    onesbd[64:128, 1] = 0.5
    xTs = [np.ascontiguousarray(x[b].T) for b in range(B)]
    for c in range(8):
        b, g = divmod(c, 4)
        j0 = 256 * g
        pm = proj[4 * g:4 * g + 4].astype(np.float32) / 8.0
        pm2 = np.zeros((2, 128, 256), dtype=np.float32)
        for p in range(2):
            pm2[p, 0:64, 0:128] = pm[2 * p].T
            pm2[p, 64:128, 128:256] = pm[2 * p + 1].T
        in_maps.append({
            "xT": xTs[b],
            "wq": np.ascontiguousarray(W_qkv[:, j0:j0 + 256]),
            "wk": np.ascontiguousarray(W_qkv[:, 1024 + j0:1024 + j0 + 256]),
            "wv": np.ascontiguousarray(W_qkv[:, 2048 + j0:2048 + j0 + 256]),
            "pm2": pm2,
            "wout": np.ascontiguousarray(W_out[j0:j0 + 256, :]).astype(
                ml_dtypes.bfloat16),
            "onesbd": onesbd,
        })
    return in_maps


def run(x, W_qkv, W_out, proj, **spmd_kwargs):
    B, N, D_ = x.shape
    in_maps = make_in_maps(np.asarray(x, dtype=np.float32),
                           np.asarray(W_qkv, dtype=np.float32),
                           np.asarray(W_out, dtype=np.float32),
                           np.asarray(proj, dtype=np.float32))
    nc = _get_nc(N)
    res = run_bass_kernel_spmd(nc, in_maps, core_ids=list(range(8)),
                               **spmd_kwargs)
    out = np.zeros((B, N, D_), dtype=np.float32)
    for c in range(8):
        b = c // 4
        out[b] += res.results[c]["y"].astype(np.float32)
    return out, res


def kernel(x, W_qkv, W_out, proj):
    x = np.asarray(x)
    assert x.shape[0] == 2 and x.shape[2] == 1024 and x.shape[1] % 512 == 0, \
        f"kernel hardcodes B=2, D=1024, N%512==0; got {x.shape}"
    out, _ = run(x, W_qkv, W_out, proj)
    return out

